# revision 21
# baseline (speedup 1.0000x reference)
"""CayleyLinear Trainium2 kernel — fp8-DoubleRow hybrid.

Computes out = x @ Q + bias where Q = (I-A)^-1 (I+A) is the Cayley
transform of the skew-symmetric matrix built from `angles`.

Strategy (data-parallel over batch, 8 NeuronCores):
  - Host: build A, solve for Q (tiny vs the 68.7 GFLOP matmul).
  - Identity+diagonal split: Q = D + R with D = diag(Q), R zero-diag.
    Device computes x @ R; the exact part x*D + bias rides in at fp16
    through a fused DVE op. This attenuates fp8 quantization noise by
    ||R||_F/sqrt(DIM) ~ 0.58.
  - Hybrid precision on the contraction: k-tiles 0..5 in fp8-e4m3 using
    DoubleRow perf mode (2 k-subtiles per matmul, 2x PE rate), k-tiles
    6..7 in fp16. Measured end-to-end rel err 1.82e-2 (gate 2e-2).
  - R is scaled by 512 before fp8/fp16 quantization so its entries
    (RMS ~0.018) land in e4m3's normal range; the drain multiplies by
    1/512 in the same fused op: out = (psum * 1/512) + (x*D + bias).
  - PE per core: 64 (m,jh) x (3 DoubleRow + 2 fp16) matmuls of 512
    moving rows = 5/8 of the pure-fp16 cycle count.
  - Output stored fp16 (rel err 2.4e-4, negligible), host casts to f32.
"""

import numpy as np

DIM = 1024
B = 8
S = 4096
N_CORES = 8
P = 128
KT8 = 6  # fp8 k-tiles (3 DoubleRow pairs)
KT16 = 2  # fp16 k-tiles
K8 = KT8 * P  # 768
RS = 512.0  # R pre-scale (power of 2; undone in the drain)

_compiled_nc = None


def _build_kernel():
    import concourse.bass as bass
    import concourse.mybir as mybir
    import concourse.tile as tile
    from concourse import bacc

    f32 = mybir.dt.float32
    f16 = mybir.dt.float16
    f8 = mybir.dt.float8e4
    DR = mybir.MatmulPerfMode.DoubleRow
    MULT = mybir.AluOpType.mult
    ADD = mybir.AluOpType.add

    nc = bacc.Bacc(
        "TRN2",
        target_bir_lowering=False,
        debug=False,
        num_devices=N_CORES,
        enable_partition_id=False,
    )

    x8_d = nc.dram_tensor("x8", [K8, S], f8, kind="ExternalInput").ap()
    x16_d = nc.dram_tensor("x16", [KT16 * P, S], f16, kind="ExternalInput").ap()
    r8_d = nc.dram_tensor("r8", [K8, DIM], f8, kind="ExternalInput").ap()
    r16_d = nc.dram_tensor("r16", [KT16 * P, DIM], f16, kind="ExternalInput").ap()
    xbd_d = nc.dram_tensor("xbd", [S, DIM], f16, kind="ExternalInput").ap()
    out_d = nc.dram_tensor("out", [S, DIM], f16, kind="ExternalOutput").ap()

    x8_r = x8_d.rearrange("(kt p) s -> p kt s", p=P)  # [128, 6, 4096]
    x16_r = x16_d.rearrange("(kt p) s -> p kt s", p=P)  # [128, 2, 4096]
    r8_r = r8_d.rearrange("(kt p) j -> p kt j", p=P)  # [128, 6, 1024]
    r16_r = r16_d.rearrange("(kt p) j -> p kt j", p=P)  # [128, 2, 1024]
    xbd_r = xbd_d.rearrange("(sb p) j -> p sb j", p=P)  # [128, 32, 1024]

    with tile.TileContext(nc) as tc:
        with (
            tc.tile_pool(name="rpool", bufs=1) as rpool,
            tc.tile_pool(name="xpool", bufs=3) as xpool,
            tc.tile_pool(name="xbpool", bufs=2) as xbpool,
            tc.tile_pool(name="opool", bufs=4) as opool,
            tc.tile_pool(name="psum", bufs=1, space="PSUM") as psumpool,
        ):
            # R tiles at per-(pair, j-half) granularity: the finest
            # chunks the matmul stream consumes, so the in-order PE
            # only ever waits on the 128-256KB chunk it needs next.
            r8_t = [
                [
                    rpool.tile([P, 2, 512], f8, name=f"r8_{kp}_{jh}")
                    for jh in range(2)
                ]
                for kp in range(KT8 // 2)
            ]
            r16_t = [
                [
                    rpool.tile([P, 512], f16, name=f"r16_{kt}_{jh}")
                    for jh in range(2)
                ]
                for kt in range(KT16)
            ]
            Copy = mybir.ActivationFunctionType.Copy

            def mm(ps, x8s, x16s, m, jh):
                # full K accumulation for one (m-block, j-half) into a
                # single [128,512] PSUM bank
                jsl = slice(jh * 512, (jh + 1) * 512)
                msl = slice(m * P, (m + 1) * P)
                for kp in range(KT8 // 2):
                    ksl = slice(2 * kp, 2 * kp + 2)
                    nc.tensor.matmul(
                        ps[:],
                        x8s[:, ksl, msl],
                        r8_t[kp][jh][:],
                        start=(kp == 0),
                        stop=False,
                        perf_mode=DR,
                    )
                for kt in range(KT16):
                    nc.tensor.matmul(
                        ps[:],
                        x16s[:, kt, msl],
                        r16_t[kt][jh][:],
                        start=False,
                        stop=(kt == KT16 - 1),
                    )

            def drain(ps, xbds, mi, sblk, jh):
                # j-half 0: scalar-ACT scales PSUM->SBUF fp16 (frees the
                # bank fast), DVE does the fp16 add.  j-half 1: one
                # fused DVE op.  Stores split sync/gpsimd.
                jsl = slice(jh * 512, (jh + 1) * 512)
                ot = opool.tile([P, 512], f16, name="ot", tag=f"ot{jh}")
                if jh == 0:
                    t16 = opool.tile([P, 512], f16, name="t16", tag="t16")
                    nc.scalar.activation(t16[:], ps[:], Copy, scale=1.0 / RS)
                    nc.vector.tensor_add(ot[:], t16[:], xbds[:, mi, jsl])
                    nc.sync.dma_start(
                        out_d[sblk * P : (sblk + 1) * P, jsl], ot[:]
                    )
                else:
                    nc.vector.scalar_tensor_tensor(
                        ot[:], ps[:], 1.0 / RS, xbds[:, mi, jsl], MULT, ADD
                    )
                    nc.gpsimd.dma_start(
                        out_d[sblk * P : (sblk + 1) * P, jsl], ot[:]
                    )

            def load_x(ssz, s_off):
                x8s = xpool.tile([P, KT8, ssz], f8, tag="x8s", name="x8s")
                nc.sync.dma_start(x8s[:], x8_r[:, :, s_off : s_off + ssz])
                x16s = xpool.tile(
                    [P, KT16, ssz], f16, tag="x16s", name="x16s"
                )
                nc.sync.dma_start(
                    x16s[:], x16_r[:, :, s_off : s_off + ssz]
                )
                return x8s, x16s

            # Warmup weights memset goes to the otherwise-idle vector
            # engine.  R chunks and the first two x slabs are issued
            # across the three DMA rings (gpsimd/scalar/sync) in the
            # exact order the matmul stream consumes them, so arrival
            # order matches need order.
            wt = rpool.tile([P, 2, 512], f8, name="wt")
            nc.vector.memset(wt[:], 0.0)
            nc.gpsimd.dma_start(r8_t[0][0][:], r8_r[:, 0:2, 0:512])
            nc.scalar.dma_start(r8_t[1][0][:], r8_r[:, 2:4, 0:512])
            pre_x = {0: load_x(128, 0)}
            nc.sync.dma_start(r8_t[2][0][:], r8_r[:, 4:6, 0:512])
            nc.gpsimd.dma_start(r16_t[0][0][:], r16_r[:, 0, 0:512])
            nc.scalar.dma_start(r16_t[1][0][:], r16_r[:, 1, 0:512])
            nc.sync.dma_start(r8_t[0][1][:], r8_r[:, 0:2, 512:])
            nc.gpsimd.dma_start(r8_t[1][1][:], r8_r[:, 2:4, 512:])
            nc.scalar.dma_start(r8_t[2][1][:], r8_r[:, 4:6, 512:])
            pre_x[1] = load_x(256, 128)
            nc.gpsimd.dma_start(r16_t[0][1][:], r16_r[:, 0, 512:])
            nc.scalar.dma_start(r16_t[1][1][:], r16_r[:, 1, 512:])
            pre_x[2] = load_x(512, 384)

            # PE warmup: throwaway DoubleRow matmuls bridge the clock
            # ramp so real matmuls start at the warm rate.
            wps = psumpool.tile([P, 512], f32, tag="ps31", name="wps")
            for _ in range(10):
                nc.tensor.matmul(
                    wps[:], wt[:, :, :P], wt[:], start=True, stop=True,
                    perf_mode=DR,
                )

            SLAB_SIZES = [128, 256] + [512] * 7 + [128]
            sblk0 = 0
            s_off = 0
            for slab, ssz in enumerate(SLAB_SIZES):
                n_m = ssz // P
                if slab in pre_x:
                    x8s, x16s = pre_x[slab]
                else:
                    x8s, x16s = load_x(ssz, s_off)
                xbds = xbpool.tile([P, n_m, DIM], f16, tag="xbd", name="xbds")
                nc.gpsimd.dma_start(
                    xbds[:], xbd_r[:, sblk0 : sblk0 + n_m, :]
                )
                pss = [
                    [
                        psumpool.tile(
                            [P, 512], f32,
                            tag=f"ps{m % 4}{jh}", name=f"ps{m % 4}{jh}",
                        )
                        for jh in range(2)
                    ]
                    for m in range(n_m)
                ]
                for m in range(n_m):
                    for jh in range(2):
                        mm(pss[m][jh], x8s, x16s, m, jh)
                        drain(pss[m][jh], xbds, m, sblk0 + m, jh)
                sblk0 += n_m
                s_off += ssz

            # PE warmdown: dummy matmuls keep the tensor engine busy
            # through the final drain+store phase so the clock doesn't
            # throttle down (HAM demotes on PE idle) while the last
            # tiles are still moving.
            wps2 = psumpool.tile([P, 512], f32, tag="ps31", name="wps2")
            for _ in range(20):
                nc.tensor.matmul(
                    wps2[:], wt[:, :, :P], wt[:], start=True, stop=True,
                    perf_mode=DR,
                )

    nc.compile()
    return nc


def _get_nc():
    global _compiled_nc
    if _compiled_nc is None:
        _compiled_nc = _build_kernel()
    return _compiled_nc


def _cayley_q(angles: np.ndarray) -> np.ndarray:
    A = np.zeros((DIM, DIM), dtype=np.float64)
    iu = np.triu_indices(DIM, k=1)
    A[iu] = angles.astype(np.float64)
    A = A - A.T
    I = np.eye(DIM, dtype=np.float64)
    return np.linalg.solve(I - A, I + A)


def _run(inputs: dict, trace: bool = False, tmpdir: str | None = None):
    import ml_dtypes
    from concourse.bass_utils import run_bass_kernel_spmd

    f8np = ml_dtypes.float8_e4m3

    x = np.asarray(inputs["x"], dtype=np.float32)
    angles = np.asarray(inputs["angles"], dtype=np.float32)
    bias = np.asarray(inputs["bias"], dtype=np.float32)

    Q = _cayley_q(angles)
    d = np.diag(Q).copy()
    R = Q - np.diag(d)  # zero diagonal
    Rs = (R * RS).astype(np.float32)
    r8 = np.ascontiguousarray(Rs[:K8]).astype(f8np)
    r16 = np.ascontiguousarray(Rs[K8:]).astype(np.float16)
    d32 = d.astype(np.float32)
    b32 = bias.astype(np.float32)

    in_maps = []
    for b in range(B):
        xT = np.ascontiguousarray(x[b].T)  # [1024, 4096] f32
        in_maps.append(
            {
                "x8": xT[:K8].astype(f8np),
                "x16": xT[K8:].astype(np.float16),
                "r8": r8,
                "r16": r16,
                "xbd": (x[b] * d32[None, :] + b32[None, :]).astype(
                    np.float16
                ),
            }
        )

    nc = _get_nc()
    res = run_bass_kernel_spmd(
        nc, in_maps, list(range(N_CORES)), trace=trace, tmpdir=tmpdir
    )
    out = np.stack(
        [res.results[b]["out"].astype(np.float32) for b in range(B)], axis=0
    )
    return out, res


def kernel(x, angles, bias):
    out, _ = _run({"x": x, "angles": angles, "bias": bias})
    return out


# revision 22
# speedup vs baseline: 1.0102x; 1.0102x over previous
"""CayleyLinear Trainium2 kernel — fp8-DoubleRow hybrid.

Computes out = x @ Q + bias where Q = (I-A)^-1 (I+A) is the Cayley
transform of the skew-symmetric matrix built from `angles`.

Strategy (data-parallel over batch, 8 NeuronCores):
  - Host: build A, solve for Q (tiny vs the 68.7 GFLOP matmul).
  - Identity+diagonal split: Q = D + R with D = diag(Q), R zero-diag.
    Device computes x @ R; the exact part x*D + bias rides in at fp16
    through a fused DVE op. This attenuates fp8 quantization noise by
    ||R||_F/sqrt(DIM) ~ 0.58.
  - Hybrid precision on the contraction: k-tiles 0..5 in fp8-e4m3 using
    DoubleRow perf mode (2 k-subtiles per matmul, 2x PE rate), k-tiles
    6..7 in fp16. Measured end-to-end rel err 1.82e-2 (gate 2e-2).
  - R is scaled by 512 before fp8/fp16 quantization so its entries
    (RMS ~0.018) land in e4m3's normal range; the drain multiplies by
    1/512 in the same fused op: out = (psum * 1/512) + (x*D + bias).
  - PE per core: 64 (m,jh) x (3 DoubleRow + 2 fp16) matmuls of 512
    moving rows = 5/8 of the pure-fp16 cycle count.
  - Output stored fp16 (rel err 2.4e-4, negligible), host casts to f32.
"""

import numpy as np

DIM = 1024
B = 8
S = 4096
N_CORES = 8
P = 128
KT8 = 6  # fp8 k-tiles (3 DoubleRow pairs)
KT16 = 2  # fp16 k-tiles
K8 = KT8 * P  # 768
RS = 512.0  # R pre-scale (power of 2; undone in the drain)

_compiled_nc = None


def _build_kernel():
    import concourse.bass as bass
    import concourse.mybir as mybir
    import concourse.tile as tile
    from concourse import bacc

    f32 = mybir.dt.float32
    f16 = mybir.dt.float16
    f8 = mybir.dt.float8e4
    DR = mybir.MatmulPerfMode.DoubleRow
    MULT = mybir.AluOpType.mult
    ADD = mybir.AluOpType.add

    nc = bacc.Bacc(
        "TRN2",
        target_bir_lowering=False,
        debug=False,
        num_devices=N_CORES,
        enable_partition_id=False,
    )

    x8_d = nc.dram_tensor("x8", [K8, S], f8, kind="ExternalInput").ap()
    x16_d = nc.dram_tensor("x16", [KT16 * P, S], f16, kind="ExternalInput").ap()
    r8_d = nc.dram_tensor("r8", [K8, DIM], f8, kind="ExternalInput").ap()
    r16_d = nc.dram_tensor("r16", [KT16 * P, DIM], f16, kind="ExternalInput").ap()
    xbd_d = nc.dram_tensor("xbd", [S, DIM], f16, kind="ExternalInput").ap()
    out_d = nc.dram_tensor("out", [S, DIM], f16, kind="ExternalOutput").ap()

    x8_r = x8_d.rearrange("(kt p) s -> p kt s", p=P)  # [128, 6, 4096]
    x16_r = x16_d.rearrange("(kt p) s -> p kt s", p=P)  # [128, 2, 4096]
    r8_r = r8_d.rearrange("(kt p) j -> p kt j", p=P)  # [128, 6, 1024]
    r16_r = r16_d.rearrange("(kt p) j -> p kt j", p=P)  # [128, 2, 1024]
    xbd_r = xbd_d.rearrange("(sb p) j -> p sb j", p=P)  # [128, 32, 1024]

    with tile.TileContext(nc) as tc:
        with (
            tc.tile_pool(name="rpool", bufs=1) as rpool,
            tc.tile_pool(name="xpool", bufs=3) as xpool,
            tc.tile_pool(name="xbpool", bufs=2) as xbpool,
            tc.tile_pool(name="opool", bufs=4) as opool,
            tc.tile_pool(name="psum", bufs=1, space="PSUM") as psumpool,
        ):
            # R tiles at per-(pair, j-half) granularity: the finest
            # chunks the matmul stream consumes, so the in-order PE
            # only ever waits on the 128-256KB chunk it needs next.
            r8_t = [
                [
                    rpool.tile([P, 2, 512], f8, name=f"r8_{kp}_{jh}")
                    for jh in range(2)
                ]
                for kp in range(KT8 // 2)
            ]
            r16_t = [
                [
                    rpool.tile([P, 512], f16, name=f"r16_{kt}_{jh}")
                    for jh in range(2)
                ]
                for kt in range(KT16)
            ]
            Copy = mybir.ActivationFunctionType.Copy

            def mm(ps, x8s, x16s, m, jh):
                # full K accumulation for one (m-block, j-half) into a
                # single [128,512] PSUM bank
                jsl = slice(jh * 512, (jh + 1) * 512)
                msl = slice(m * P, (m + 1) * P)
                for kp in range(KT8 // 2):
                    ksl = slice(2 * kp, 2 * kp + 2)
                    nc.tensor.matmul(
                        ps[:],
                        x8s[:, ksl, msl],
                        r8_t[kp][jh][:],
                        start=(kp == 0),
                        stop=False,
                        perf_mode=DR,
                    )
                for kt in range(KT16):
                    nc.tensor.matmul(
                        ps[:],
                        x16s[:, kt, msl],
                        r16_t[kt][jh][:],
                        start=False,
                        stop=(kt == KT16 - 1),
                    )

            def drain(ps, xbds, mi, sblk, jh):
                # j-half 0: scalar-ACT scales PSUM->SBUF fp16 (frees the
                # bank fast), DVE does the fp16 add.  j-half 1: one
                # fused DVE op.  Stores split sync/gpsimd.
                jsl = slice(jh * 512, (jh + 1) * 512)
                ot = opool.tile([P, 512], f16, name="ot", tag=f"ot{jh}")
                if jh == 0:
                    t16 = opool.tile([P, 512], f16, name="t16", tag="t16")
                    nc.scalar.activation(t16[:], ps[:], Copy, scale=1.0 / RS)
                    nc.vector.tensor_add(ot[:], t16[:], xbds[:, mi, jsl])
                    nc.sync.dma_start(
                        out_d[sblk * P : (sblk + 1) * P, jsl], ot[:]
                    )
                else:
                    nc.vector.scalar_tensor_tensor(
                        ot[:], ps[:], 1.0 / RS, xbds[:, mi, jsl], MULT, ADD
                    )
                    nc.gpsimd.dma_start(
                        out_d[sblk * P : (sblk + 1) * P, jsl], ot[:]
                    )

            def load_x(ssz, s_off):
                x8s = xpool.tile([P, KT8, ssz], f8, tag="x8s", name="x8s")
                nc.sync.dma_start(x8s[:], x8_r[:, :, s_off : s_off + ssz])
                x16s = xpool.tile(
                    [P, KT16, ssz], f16, tag="x16s", name="x16s"
                )
                nc.sync.dma_start(
                    x16s[:], x16_r[:, :, s_off : s_off + ssz]
                )
                return x8s, x16s

            # Warmup weights memset goes to the otherwise-idle vector
            # engine.  R chunks and the first two x slabs are issued
            # across the three DMA rings (gpsimd/scalar/sync) in the
            # exact order the matmul stream consumes them, so arrival
            # order matches need order.
            wt = rpool.tile([P, 2, 512], f8, name="wt")
            nc.vector.memset(wt[:], 0.0)
            nc.gpsimd.dma_start(r8_t[0][0][:], r8_r[:, 0:2, 0:512])
            nc.scalar.dma_start(r8_t[1][0][:], r8_r[:, 2:4, 0:512])
            pre_x = {0: load_x(128, 0)}
            nc.sync.dma_start(r8_t[2][0][:], r8_r[:, 4:6, 0:512])
            nc.gpsimd.dma_start(r16_t[0][0][:], r16_r[:, 0, 0:512])
            nc.scalar.dma_start(r16_t[1][0][:], r16_r[:, 1, 0:512])
            nc.sync.dma_start(r8_t[0][1][:], r8_r[:, 0:2, 512:])
            nc.gpsimd.dma_start(r8_t[1][1][:], r8_r[:, 2:4, 512:])
            nc.scalar.dma_start(r8_t[2][1][:], r8_r[:, 4:6, 512:])
            pre_x[1] = load_x(256, 128)
            nc.gpsimd.dma_start(r16_t[0][1][:], r16_r[:, 0, 512:])
            nc.scalar.dma_start(r16_t[1][1][:], r16_r[:, 1, 512:])
            pre_x[2] = load_x(512, 384)

            # PE warmup: throwaway DoubleRow matmuls bridge the clock
            # ramp so real matmuls start at the warm rate.
            wps = psumpool.tile([P, 512], f32, tag="ps31", name="wps")
            for _ in range(10):
                nc.tensor.matmul(
                    wps[:], wt[:, :, :P], wt[:], start=True, stop=True,
                    perf_mode=DR,
                )

            SLAB_SIZES = [128, 256] + [512] * 7 + [128]
            sblk0 = 0
            s_off = 0
            for slab, ssz in enumerate(SLAB_SIZES):
                n_m = ssz // P
                if slab in pre_x:
                    x8s, x16s = pre_x[slab]
                else:
                    x8s, x16s = load_x(ssz, s_off)
                xbds = xbpool.tile([P, n_m, DIM], f16, tag="xbd", name="xbds")
                nc.gpsimd.dma_start(
                    xbds[:], xbd_r[:, sblk0 : sblk0 + n_m, :]
                )
                pss = [
                    [
                        psumpool.tile(
                            [P, 512], f32,
                            tag=f"ps{m % 4}{jh}", name=f"ps{m % 4}{jh}",
                        )
                        for jh in range(2)
                    ]
                    for m in range(n_m)
                ]
                for m in range(n_m):
                    for jh in range(2):
                        mm(pss[m][jh], x8s, x16s, m, jh)
                        drain(pss[m][jh], xbds, m, sblk0 + m, jh)
                sblk0 += n_m
                s_off += ssz



    nc.compile()
    return nc


def _get_nc():
    global _compiled_nc
    if _compiled_nc is None:
        _compiled_nc = _build_kernel()
    return _compiled_nc


def _cayley_q(angles: np.ndarray) -> np.ndarray:
    A = np.zeros((DIM, DIM), dtype=np.float64)
    iu = np.triu_indices(DIM, k=1)
    A[iu] = angles.astype(np.float64)
    A = A - A.T
    I = np.eye(DIM, dtype=np.float64)
    return np.linalg.solve(I - A, I + A)


def _run(inputs: dict, trace: bool = False, tmpdir: str | None = None):
    import ml_dtypes
    from concourse.bass_utils import run_bass_kernel_spmd

    f8np = ml_dtypes.float8_e4m3

    x = np.asarray(inputs["x"], dtype=np.float32)
    angles = np.asarray(inputs["angles"], dtype=np.float32)
    bias = np.asarray(inputs["bias"], dtype=np.float32)

    Q = _cayley_q(angles)
    d = np.diag(Q).copy()
    R = Q - np.diag(d)  # zero diagonal
    Rs = (R * RS).astype(np.float32)
    r8 = np.ascontiguousarray(Rs[:K8]).astype(f8np)
    r16 = np.ascontiguousarray(Rs[K8:]).astype(np.float16)
    d32 = d.astype(np.float32)
    b32 = bias.astype(np.float32)

    in_maps = []
    for b in range(B):
        xT = np.ascontiguousarray(x[b].T)  # [1024, 4096] f32
        in_maps.append(
            {
                "x8": xT[:K8].astype(f8np),
                "x16": xT[K8:].astype(np.float16),
                "r8": r8,
                "r16": r16,
                "xbd": (x[b] * d32[None, :] + b32[None, :]).astype(
                    np.float16
                ),
            }
        )

    nc = _get_nc()
    res = run_bass_kernel_spmd(
        nc, in_maps, list(range(N_CORES)), trace=trace, tmpdir=tmpdir
    )
    out = np.stack(
        [res.results[b]["out"].astype(np.float32) for b in range(B)], axis=0
    )
    return out, res


def kernel(x, angles, bias):
    out, _ = _run({"x": x, "angles": angles, "bias": bias})
    return out
